# revision 56
# baseline (speedup 1.0000x reference)
"""BertBiAttention Trainium2 kernel.

Cross-attention between two streams (B=4, S=2048, HID=768, H=12 heads).
Sharding: 8 cores = (stream s in {1,2}) x (batch b in {0..3}). Each core
computes one stream's full output for one batch element:
    h_s[b] = LayerNorm( attend(q_other, k_own, v_own, mask_own) @ wd + bd + x_own )
No collectives needed; the host stacks per-core outputs.

On-chip layouts (per core):
  qT, kT  [768, 2048] bf16  (feature-major, head h at partition rows h*64..)
  v       [2048 (16x128), 12, 65] bf16  (per head: [v*emask | emask] columns;
          odd heads store [emask | v*emask] so the PSUM partition ranges of
          the normalization never cross the 64-lane boundary)
  scoresT [krows, q] in PSUM -> exp (ACT, scale=1/8) -> bf16
  ctxT    accumulated via lhsT=[v|1] matmuls; row 64 (or 63) = softmax denom
  dense   h = ctxT.T @ wd (+bd via K=1 ones matmul) + residual, LayerNorm.
All matmuls fp32r (full inputs) or bf16 (attention path); PSUM accum fp32.
"""

import numpy as np

import concourse.bass as bass
import concourse.mybir as mybir
import concourse.tile as tile
from concourse import bacc, bass_utils
from concourse.masks import make_identity

B, S, HID, H, HD = 4, 2048, 768, 12, 64
FT = HID // 128   # 6 feature tiles
ST = S // 128     # 16 seq tiles
QT = S // 512     # 4 q chunks
NH = 2            # 768-wide outputs split into 2 x 384
NW = 384
EPS = 1e-12

F32 = mybir.dt.float32
F32R = mybir.dt.float32r
BF16 = mybir.dt.bfloat16
AF = mybir.ActivationFunctionType


def _bcast_part(ap, p=128):
    """DRAM row [1, N] -> partition-broadcast AP [p, N] (stride-0 partition)."""
    return bass.AP(tensor=ap.tensor, offset=ap.offset, ap=[[0, p], ap.ap[-1]])


def _setup_act_tables():
    """Point the compiler at an act_info.json whose first set covers both
    exp and ln (natural_log_exp_and_others), so the kernel's Exp and Ln
    activations share one ACT table set instead of reloading (~1.3us) on
    every switch."""
    import json
    import os
    import tempfile
    from pathlib import Path

    if os.environ.get("BASS_ACT_ROOT_JSON_PATH"):
        return
    try:
        from neuronxcc.driver.Job import Job
        from neuronxcc.driver.jobs.support.FindActInfo import findActInfoFile

        src = Path(findActInfoFile(Job.getPackageDir(), "gen3"))
        d = json.loads(src.read_text())
        sets = d["act_func_sets"]
        pref = [s for s in sets if s["name"] == "natural_log_exp_and_others"]
        rest = [s for s in sets if s["name"] != "natural_log_exp_and_others"]
        if not pref:
            return
        d["act_func_sets"] = pref + rest
        dst = Path(tempfile.mkdtemp(prefix="act_tables_"))
        for f in src.parent.iterdir():
            if f.name != src.name and f.is_file():
                os.symlink(f, dst / f.name)
        (dst / src.name).write_text(json.dumps(d))
        os.environ["BASS_ACT_ROOT_JSON_PATH"] = str(dst / src.name)
    except Exception:
        pass  # default tables still work, just slower


def build_nc():
    # _setup_act_tables()  # crashes the exec unit via this compile path
    nc = bacc.Bacc("TRN2", target_bir_lowering=False, debug=False, num_devices=8)

    xq_d = nc.dram_tensor("xq", [S, HID], F32, kind="ExternalInput").ap()
    xkv_d = nc.dram_tensor("xkv", [S, HID], F32, kind="ExternalInput").ap()
    wq_d = nc.dram_tensor("wq", [HID, HID], F32, kind="ExternalInput").ap()
    wk_d = nc.dram_tensor("wk", [HID, HID], F32, kind="ExternalInput").ap()
    wv_d = nc.dram_tensor("wv", [HID, HID], F32, kind="ExternalInput").ap()
    wd_d = nc.dram_tensor("wd", [HID, HID], F32, kind="ExternalInput").ap()
    bq_d = nc.dram_tensor("bq", [1, HID], F32, kind="ExternalInput").ap()
    bk_d = nc.dram_tensor("bk", [1, HID], F32, kind="ExternalInput").ap()
    bv_d = nc.dram_tensor("bv", [1, HID], F32, kind="ExternalInput").ap()
    bd_d = nc.dram_tensor("bd", [1, HID], F32, kind="ExternalInput").ap()
    mask_d = nc.dram_tensor("mask", [S, 1], F32, kind="ExternalInput").ap()
    lng_d = nc.dram_tensor("lng", [1, HID], F32, kind="ExternalInput").ap()
    lnb_d = nc.dram_tensor("lnb", [1, HID], F32, kind="ExternalInput").ap()
    out_d = nc.dram_tensor("out", [S, HID], F32, kind="ExternalOutput").ap()

    with tile.TileContext(nc) as tc:
        with (
            tc.tile_pool(name="consts", bufs=1) as consts,
            tc.tile_pool(name="big", bufs=1) as big,
        ):
            # ---- constants ----
            ident = consts.tile([128, 128], F32)
            make_identity(nc, ident)
            ones_r = consts.tile([1, 128], BF16)
            nc.vector.memset(ones_r, 1.0)
            ones_12 = consts.tile([128, 12], F32)
            nc.vector.memset(ones_12, 1.0)
            eps_t = consts.tile([128, 1], F32)
            nc.vector.memset(eps_t, EPS)

            bqc = consts.tile([128, FT], F32)
            bkc = consts.tile([128, FT], F32)
            for f in range(FT):
                nc.sync.dma_start(
                    out=bqc[:, f : f + 1],
                    in_=bq_d[0:1, f * 128 : (f + 1) * 128].rearrange("a b -> b a"),
                )
                nc.sync.dma_start(
                    out=bkc[:, f : f + 1],
                    in_=bk_d[0:1, f * 128 : (f + 1) * 128].rearrange("a b -> b a"),
                )
            bv_f = consts.tile([1, HID], F32)
            nc.sync.dma_start(out=bv_f, in_=bv_d)
            bd_f = consts.tile([1, HID], F32)
            nc.sync.dma_start(out=bd_f, in_=bd_d)
            bv_row = consts.tile([1, HID], BF16)
            nc.vector.tensor_copy(out=bv_row, in_=bv_f)
            bd_row = consts.tile([1, HID], BF16)
            nc.vector.tensor_copy(out=bd_row, in_=bd_f)

            mask_t = consts.tile([128, ST], F32)
            for t in range(ST):
                nc.sync.dma_start(
                    out=mask_t[:, t : t + 1], in_=mask_d[t * 128 : (t + 1) * 128, :]
                )
            emask = consts.tile([128, ST], F32)
            nc.scalar.activation(out=emask, in_=mask_t, func=AF.Exp)

            # broadcast ln gamma/beta to all 128 partitions (stride-0 DMA)
            g_bc = consts.tile([128, HID], F32)
            b_bc = consts.tile([128, HID], F32)
            nc.sync.dma_start(out=g_bc, in_=_bcast_part(lng_d))
            nc.sync.dma_start(out=b_bc, in_=_bcast_part(lnb_d))

            # ---- persistent activation buffers ----
            qT = [big.tile([128, S], BF16, name=f"qT{f}") for f in range(FT)]
            kT = [big.tile([128, S], BF16, name=f"kT{f}") for f in range(FT)]
            vb = [big.tile([128, H, HD + 1], BF16, name=f"vb{t}") for t in range(ST)]
            # wd stored per-head ([64, 768] at partition base 0) so the dense
            # per-head K=64 matmuls have base-aligned lhsT/rhs
            dw_bf = [big.tile([HD, HID], BF16, name=f"dwbf{h}") for h in range(H)]

            # ---- projections ----
            def project_chunk(x_d, xT_c, ps_tp, xn_pool, chunk):
                """DMA 512 rows of x, transpose into xT_c [128, FT, 512] f32."""
                for ss in range(4):
                    x_nat = xn_pool.tile([128, HID], F32, name="x_nat")
                    st = chunk * 4 + ss
                    nc.sync.dma_start(
                        out=x_nat, in_=x_d[st * 128 : (st + 1) * 128, :]
                    )
                    for f in range(FT):
                        tp_ps = ps_tp.tile([128, 128], F32, name="tp_ps")
                        nc.tensor.transpose(
                            tp_ps, x_nat[:, f * 128 : (f + 1) * 128], ident
                        )
                        nc.vector.tensor_copy(
                            out=xT_c[:, f, ss * 128 : (ss + 1) * 128], in_=tp_ps
                        )

            with (
                tc.tile_pool(name="wq_pool", bufs=1) as wq_pool,
                tc.tile_pool(name="xn", bufs=3) as xn_pool,
                tc.tile_pool(name="xT", bufs=2) as xT_pool,
                tc.tile_pool(name="ps_tp", bufs=4, space="PSUM") as ps_tp,
                tc.tile_pool(name="ps_pj", bufs=2, space="PSUM") as ps_pj,
            ):
                wq_b = [
                    wq_pool.tile([128, HID], BF16, name=f"wq{f}") for f in range(FT)
                ]
                for f in range(FT):
                    wtmp = xn_pool.tile([128, HID], F32, name="wtmp")
                    nc.sync.dma_start(out=wtmp, in_=wq_d[f * 128 : (f + 1) * 128, :])
                    nc.vector.tensor_copy(out=wq_b[f], in_=wtmp)
                # load wd (fp32) per head and convert to bf16
                for h in range(H):
                    wd_t = xn_pool.tile([HD, HID], F32, name="wd_t")
                    nc.sync.dma_start(out=wd_t, in_=wd_d[h * HD : (h + 1) * HD, :])
                    nc.vector.tensor_copy(out=dw_bf[h], in_=wd_t)

                for chunk in range(QT):
                    xT_c = xT_pool.tile([128, FT, 512], BF16, name="xT_q")
                    project_chunk(xq_d, xT_c, ps_tp, xn_pool, chunk)
                    for fo in range(FT):
                        pj = ps_pj.tile([128, 512], F32, name="pj")
                        for kf in range(FT):
                            nc.tensor.matmul(
                                pj,
                                wq_b[kf][:, fo * 128 : (fo + 1) * 128],
                                xT_c[:, kf, :],
                                start=(kf == 0),
                                stop=(kf == FT - 1),
                            )
                        nc.vector.tensor_scalar_add(
                            out=qT[fo][:, chunk * 512 : (chunk + 1) * 512],
                            in0=pj,
                            scalar1=bqc[:, fo : fo + 1],
                        )

            with (
                tc.tile_pool(name="wkv_pool", bufs=1) as wkv_pool,
                tc.tile_pool(name="xn2", bufs=3) as xn2_pool,
                tc.tile_pool(name="xT2", bufs=2) as xT2_pool,
                tc.tile_pool(name="ps_tp2", bufs=2, space="PSUM") as ps_tp2,
                tc.tile_pool(name="ps_pj2", bufs=2, space="PSUM") as ps_pj2,
                tc.tile_pool(name="ps_v", bufs=2, space="PSUM") as ps_v,
            ):
                wk_b = [
                    wkv_pool.tile([128, HID], BF16, name=f"wk{f}") for f in range(FT)
                ]
                wv_b = [
                    wkv_pool.tile([128, HID], BF16, name=f"wv{f}") for f in range(FT)
                ]
                for f in range(FT):
                    wtmp = xn2_pool.tile([128, HID], F32, name="wtmp2")
                    nc.sync.dma_start(out=wtmp, in_=wk_d[f * 128 : (f + 1) * 128, :])
                    nc.vector.tensor_copy(out=wk_b[f], in_=wtmp)
                    wtmp = xn2_pool.tile([128, HID], F32, name="wtmp2")
                    nc.sync.dma_start(out=wtmp, in_=wv_d[f * 128 : (f + 1) * 128, :])
                    nc.vector.tensor_copy(out=wv_b[f], in_=wtmp)

                for chunk in range(QT):
                    xT_c = xT2_pool.tile([128, FT, 512], BF16, name="xT_kv")
                    project_chunk(xkv_d, xT_c, ps_tp2, xn2_pool, chunk)
                    # kT
                    for fo in range(FT):
                        pj = ps_pj2.tile([128, 512], F32, name="pj2")
                        for kf in range(FT):
                            nc.tensor.matmul(
                                pj,
                                wk_b[kf][:, fo * 128 : (fo + 1) * 128],
                                xT_c[:, kf, :],
                                start=(kf == 0),
                                stop=(kf == FT - 1),
                            )
                        nc.vector.tensor_scalar_add(
                            out=kT[fo][:, chunk * 512 : (chunk + 1) * 512],
                            in0=pj,
                            scalar1=bkc[:, fo : fo + 1],
                        )
                    # v (natural layout, rows scaled by exp(mask), + denom col)
                    for ss in range(4):
                        st = chunk * 4 + ss
                        vp = ps_v.tile([128, NH, 512], F32, name="vp")
                        for nh in range(NH):
                            for kf in range(FT):
                                nc.tensor.matmul(
                                    vp[:, nh, 0:NW],
                                    xT_c[:, kf, ss * 128 : (ss + 1) * 128],
                                    wv_b[kf][:, nh * NW : (nh + 1) * NW],
                                    start=(kf == 0),
                                    stop=False,
                                )
                            nc.tensor.matmul(
                                vp[:, nh, 0:NW],
                                ones_r,
                                bv_row[0:1, nh * NW : (nh + 1) * NW],
                                start=False,
                                stop=True,
                            )
                        emcol = emask[:, st : st + 1]
                        for nh in range(NH):
                            nc.vector.tensor_scalar_mul(
                                out=vb[st][:, nh * 6 : (nh + 1) * 6, 0:HD],
                                in0=vp[:, nh, 0:NW].rearrange(
                                    "p (a d) -> p a d", a=6
                                ),
                                scalar1=emcol,
                            )
                        nc.vector.tensor_scalar_mul(
                            out=vb[st][:, :, HD : HD + 1].rearrange(
                                "p a c -> p (a c)"
                            ),
                            in0=ones_12,
                            scalar1=emcol,
                        )

            # ---- attention + dense + layernorm, per 512-wide q chunk ----
            with (
                tc.tile_pool(name="ctx_pool", bufs=2) as ctx_pool,
                tc.tile_pool(name="ctxu_pool", bufs=1) as ctxu_pool,
                tc.tile_pool(name="dram_pool", bufs=2, space="DRAM") as dram_pool,
                tc.tile_pool(name="exp_pool", bufs=4) as exp_pool,
                tc.tile_pool(name="rec_pool", bufs=2) as rec_pool,
                tc.tile_pool(name="res_pool", bufs=3) as res_pool,
                tc.tile_pool(name="hpre_pool", bufs=2) as hpre_pool,
                tc.tile_pool(name="st_pool", bufs=4) as st_pool,
                tc.tile_pool(name="ps_sc", bufs=2, space="PSUM") as ps_sc,
                tc.tile_pool(name="ps_ctx", bufs=2, space="PSUM") as ps_ctx,
                tc.tile_pool(name="ps_h", bufs=2, space="PSUM") as ps_h,
            ):
                def make_dense_steps(qt, ctx_t):
                    """Dense + residual + LN for chunk qt as 9 deferred steps,
                    emitted between the next chunk's attention heads so the
                    in-order PE has fill work while ACT computes exps."""
                    state = {}

                    def group_step(ss, nh):
                        def run():
                            if "mvq" not in state:
                                state["mvq"] = st_pool.tile(
                                    [128, 4, 2], F32, name="mvq"
                                )
                                state["hp"] = {}
                            st = qt * 4 + ss
                            ssl = slice(ss * 128, (ss + 1) * 128)
                            if ss not in state["hp"]:
                                state["hp"][ss] = hpre_pool.tile(
                                    [128, HID], F32, name=f"hp{ss}"
                                )
                            hp = state["hp"][ss]
                            h_ps = ps_h.tile([128, 512], F32, name="h_ps")
                            for hh in range(H):
                                nc.tensor.matmul(
                                    h_ps[:, 0:NW],
                                    ctx_t[hh][:, ssl],
                                    dw_bf[hh][:, nh * NW : (nh + 1) * NW],
                                    start=(hh == 0),
                                    stop=False,
                                )
                            nc.tensor.matmul(
                                h_ps[:, 0:NW],
                                ones_r,
                                bd_row[0:1, nh * NW : (nh + 1) * NW],
                                start=False,
                                stop=True,
                            )
                            x_res = res_pool.tile([128, NW], F32, name="x_res")
                            nc.sync.dma_start(
                                out=x_res,
                                in_=xkv_d[
                                    st * 128 : (st + 1) * 128,
                                    nh * NW : (nh + 1) * NW,
                                ],
                            )
                            nc.vector.tensor_add(
                                out=hp[:, nh * NW : (nh + 1) * NW],
                                in0=h_ps[:, 0:NW],
                                in1=x_res,
                            )
                            if nh == NH - 1:
                                stats = st_pool.tile([128, 3, 6], F32, name="stats")
                                for sg in range(3):
                                    nc.vector.bn_stats(
                                        out=stats[:, sg, :],
                                        in_=hp[:, sg * 256 : (sg + 1) * 256],
                                    )
                                nc.vector.bn_aggr(
                                    out=state["mvq"][:, ss, :], in_=stats
                                )

                        return run

                    def tail():
                        mvq = state["mvq"]
                        # rstd = exp(-0.5*ln(var+eps)), batched over the 4
                        # subtiles (2 ACT table switches per chunk)
                        lnv = st_pool.tile([128, 4], F32, name="lnv")
                        nc.scalar.activation(
                            out=lnv, in_=mvq[:, :, 1], func=AF.Ln,
                            bias=eps_t, scale=1.0,
                        )
                        rstd4 = st_pool.tile([128, 4], F32, name="rstd4")
                        nc.scalar.activation(
                            out=rstd4, in_=lnv, func=AF.Exp, scale=-0.5
                        )
                        for ss in range(4):
                            st = qt * 4 + ss
                            hp = state["hp"][ss]
                            hn = hpre_pool.tile([128, HID], F32, name="hn")
                            nc.vector.tensor_scalar(
                                out=hn,
                                in0=hp,
                                scalar1=mvq[:, ss, 0:1],
                                scalar2=rstd4[:, ss : ss + 1],
                                op0=mybir.AluOpType.subtract,
                                op1=mybir.AluOpType.mult,
                            )
                            nc.vector.tensor_mul(hn, hn, g_bc)
                            nc.vector.tensor_add(hn, hn, b_bc)
                            nc.sync.dma_start(
                                out=out_d[st * 128 : (st + 1) * 128, :], in_=hn
                            )

                    return [group_step(ss, nh) for ss in range(4) for nh in range(NH)] + [tail]

                pending = []
                for qt in range(QT):
                    ctx_t = [
                        ctx_pool.tile([HD, 512], BF16, name=f"ctx{h}")
                        for h in range(H)
                    ]
                    ctxu = [
                        ctxu_pool.tile([HD, 512], BF16, name=f"ctxu{h}")
                        for h in range(H)
                    ]
                    den_all = rec_pool.tile([H, 512], F32, name="den_all")
                    qsl = slice(qt * 512, (qt + 1) * 512)
                    for h in range(H):
                        ft, po = h // 2, (h % 2) * 64
                        ctx_ps = ps_ctx.tile([HD + 1, 512], F32, name="ctx_ps")
                        # software-pipelined: scores(g) are emitted before
                        # ctx(g-1) so the in-order PE streams scores while
                        # ACT computes exp(g-1) — no per-group PE stall
                        exps = []
                        for g in range(8):
                            sc_ps = ps_sc.tile([128, 2, 512], F32, name="sc_ps")
                            for j in range(2):
                                kc = g * 2 + j
                                nc.tensor.matmul(
                                    sc_ps[:, j, :],
                                    kT[ft][po : po + HD, kc * 128 : (kc + 1) * 128],
                                    qT[ft][po : po + HD, qsl],
                                    start=True,
                                    stop=True,
                                )
                            exp_g = exp_pool.tile([128, 2, 512], BF16, name="exp_g")
                            nc.scalar.activation(
                                out=exp_g, in_=sc_ps, func=AF.Exp, scale=0.125
                            )
                            exps.append(exp_g)
                            if g in (2, 5) and pending:
                                pending.pop(0)()  # fill PE while ACT works
                            if g > 0:
                                for j in range(2):
                                    kc = (g - 1) * 2 + j
                                    nc.tensor.matmul(
                                        ctx_ps,
                                        vb[kc][:, h, :],
                                        exps[g - 1][:, j, :],
                                        start=(g == 1 and j == 0),
                                        stop=False,
                                    )
                        for j in range(2):
                            kc = 7 * 2 + j
                            nc.tensor.matmul(
                                ctx_ps,
                                vb[kc][:, h, :],
                                exps[7][:, j, :],
                                start=False,
                                stop=(j == 1),
                            )
                        # evict unnormalized ctx + denominator; normalization
                        # is batched per chunk (below), off the head loop
                        dtmp = rec_pool.tile([HD + 1, 512], F32, name="dtmp")
                        nc.vector.tensor_copy(
                            out=dtmp[HD : HD + 1, :], in_=ctx_ps[HD : HD + 1, :]
                        )
                        nc.sync.dma_start(
                            out=den_all[h : h + 1, :], in_=dtmp[HD : HD + 1, :]
                        )
                        nc.vector.tensor_copy(out=ctxu[h], in_=ctx_ps[0:HD, :])
                    # one iterative-divide pass for all 12 heads' denominators
                    rec_all = rec_pool.tile([H, 512], F32, name="rec_all")
                    nc.vector.reciprocal(rec_all, den_all)
                    # partition-broadcast needs a DRAM source: bounce the
                    # reciprocal rows out, then stride-0 DMA back per head
                    rec_d = dram_pool.tile([H, 512], F32, name="rec_d")
                    nc.sync.dma_start(out=rec_d, in_=rec_all)
                    for h in range(H):
                        bc_sb = rec_pool.tile([HD, 512], F32, name="bc_sb")
                        nc.sync.dma_start(
                            out=bc_sb,
                            in_=rec_d[h : h + 1, :].to_broadcast((HD, 512)),
                        )
                        nc.vector.tensor_mul(
                            out=ctx_t[h], in0=ctxu[h], in1=bc_sb
                        )
                        if pending and h % 2 == 0:
                            pending.pop(0)()

                    pending = make_dense_steps(qt, ctx_t)
                for step in pending:
                    step()

    nc.compile()
    return nc


_NC = None


def _get_nc():
    global _NC
    if _NC is None:
        _NC = build_nc()
    return _NC


def _prepare(
    input_tensor1, attention_mask1, input_tensor2, attention_mask2,
    q1_w, q1_b, k1_w, k1_b, v1_w, v1_b,
    q2_w, q2_b, k2_w, k2_b, v2_w, v2_b,
    d1_w, d1_b, d2_w, d2_b, ln1_g, ln1_b, ln2_g, ln2_b,
):
    f = lambda a: np.ascontiguousarray(np.asarray(a), dtype=np.float32)
    x1, x2 = f(input_tensor1), f(input_tensor2)
    m1 = f(attention_mask1).reshape(B, S, 1)
    m2 = f(attention_mask2).reshape(B, S, 1)
    row = lambda a: f(a).reshape(1, HID)

    in_maps = []
    for b in range(B):
        # stream1: ctx1 = attend(q2, k1, v1, mask1); out h1[b]
        in_maps.append({
            "xq": x2[b], "xkv": x1[b],
            "wq": f(q2_w), "wk": f(k1_w), "wv": f(v1_w), "wd": f(d1_w),
            "bq": row(q2_b), "bk": row(k1_b), "bv": row(v1_b), "bd": row(d1_b),
            "mask": m1[b], "lng": row(ln1_g), "lnb": row(ln1_b),
        })
    for b in range(B):
        # stream2: ctx2 = attend(q1, k2, v2, mask2); out h2[b]
        in_maps.append({
            "xq": x1[b], "xkv": x2[b],
            "wq": f(q1_w), "wk": f(k2_w), "wv": f(v2_w), "wd": f(d2_w),
            "bq": row(q1_b), "bk": row(k2_b), "bv": row(v2_b), "bd": row(d2_b),
            "mask": m2[b], "lng": row(ln2_g), "lnb": row(ln2_b),
        })

    return in_maps


def _run(in_maps, **kwargs):
    nc = _get_nc()
    res = bass_utils.run_bass_kernel_spmd(
        nc, in_maps, core_ids=list(range(8)), **kwargs
    )
    h1 = np.stack([res.results[b]["out"] for b in range(B)])
    h2 = np.stack([res.results[B + b]["out"] for b in range(B)])
    return (h1, h2), res


def kernel(**inputs):
    (h1, h2), _ = _run(_prepare(**inputs))
    return h1, h2


# revision 57
# speedup vs baseline: 1.1370x; 1.1370x over previous
"""BertBiAttention Trainium2 kernel.

Cross-attention between two streams (B=4, S=2048, HID=768, H=12 heads).
Sharding: 8 cores = (stream s in {1,2}) x (batch b in {0..3}). Each core
computes one stream's full output for one batch element:
    h_s[b] = LayerNorm( attend(q_other, k_own, v_own, mask_own) @ wd + bd + x_own )
No collectives needed; the host stacks per-core outputs.

On-chip layouts (per core):
  qT, kT  [768, 2048] bf16  (feature-major, head h at partition rows h*64..)
  v       [2048 (16x128), 12, 65] bf16  (per head: [v*emask | emask] columns;
          odd heads store [emask | v*emask] so the PSUM partition ranges of
          the normalization never cross the 64-lane boundary)
  scoresT [krows, q] in PSUM -> exp (ACT, scale=1/8) -> bf16
  ctxT    accumulated via lhsT=[v|1] matmuls; row 64 (or 63) = softmax denom
  dense   h = ctxT.T @ wd (+bd via K=1 ones matmul) + residual, LayerNorm.
All matmuls fp32r (full inputs) or bf16 (attention path); PSUM accum fp32.
"""

import numpy as np

import concourse.bass as bass
import concourse.mybir as mybir
import concourse.tile as tile
from concourse import bacc, bass_utils
from concourse.masks import make_identity

B, S, HID, H, HD = 4, 2048, 768, 12, 64
FT = HID // 128   # 6 feature tiles
ST = S // 128     # 16 seq tiles
QT = S // 512     # 4 q chunks
NH = 2            # 768-wide outputs split into 2 x 384
NW = 384
EPS = 1e-12

F32 = mybir.dt.float32
F32R = mybir.dt.float32r
BF16 = mybir.dt.bfloat16
AF = mybir.ActivationFunctionType


def _bcast_part(ap, p=128):
    """DRAM row [1, N] -> partition-broadcast AP [p, N] (stride-0 partition)."""
    return bass.AP(tensor=ap.tensor, offset=ap.offset, ap=[[0, p], ap.ap[-1]])


def _setup_act_tables():
    """Point the compiler at an act_info.json whose first set covers both
    exp and ln (natural_log_exp_and_others), so the kernel's Exp and Ln
    activations share one ACT table set instead of reloading (~1.3us) on
    every switch."""
    import json
    import os
    import tempfile
    from pathlib import Path

    if os.environ.get("BASS_ACT_ROOT_JSON_PATH"):
        return
    try:
        from neuronxcc.driver.Job import Job
        from neuronxcc.driver.jobs.support.FindActInfo import findActInfoFile

        src = Path(findActInfoFile(Job.getPackageDir(), "gen3"))
        d = json.loads(src.read_text())
        sets = d["act_func_sets"]
        pref = [s for s in sets if s["name"] == "natural_log_exp_and_others"]
        rest = [s for s in sets if s["name"] != "natural_log_exp_and_others"]
        if not pref:
            return
        d["act_func_sets"] = pref + rest
        dst = Path(tempfile.mkdtemp(prefix="act_tables_"))
        for f in src.parent.iterdir():
            if f.name != src.name and f.is_file():
                os.symlink(f, dst / f.name)
        (dst / src.name).write_text(json.dumps(d))
        os.environ["BASS_ACT_ROOT_JSON_PATH"] = str(dst / src.name)
    except Exception:
        pass  # default tables still work, just slower


def build_nc():
    # _setup_act_tables()  # crashes the exec unit via this compile path
    nc = bacc.Bacc("TRN2", target_bir_lowering=False, debug=False, num_devices=8)

    xq_d = nc.dram_tensor("xq", [S, HID], F32, kind="ExternalInput").ap()
    xkv_d = nc.dram_tensor("xkv", [S, HID], F32, kind="ExternalInput").ap()
    wq_d = nc.dram_tensor("wq", [HID, HID], F32, kind="ExternalInput").ap()
    wk_d = nc.dram_tensor("wk", [HID, HID], F32, kind="ExternalInput").ap()
    wv_d = nc.dram_tensor("wv", [HID, HID], F32, kind="ExternalInput").ap()
    wd_d = nc.dram_tensor("wd", [HID, HID], F32, kind="ExternalInput").ap()
    bq_d = nc.dram_tensor("bq", [1, HID], F32, kind="ExternalInput").ap()
    bk_d = nc.dram_tensor("bk", [1, HID], F32, kind="ExternalInput").ap()
    bv_d = nc.dram_tensor("bv", [1, HID], F32, kind="ExternalInput").ap()
    bd_d = nc.dram_tensor("bd", [1, HID], F32, kind="ExternalInput").ap()
    mask_d = nc.dram_tensor("mask", [S, 1], F32, kind="ExternalInput").ap()
    lng_d = nc.dram_tensor("lng", [1, HID], F32, kind="ExternalInput").ap()
    lnb_d = nc.dram_tensor("lnb", [1, HID], F32, kind="ExternalInput").ap()
    out_d = nc.dram_tensor("out", [S, HID], F32, kind="ExternalOutput").ap()

    with tile.TileContext(nc) as tc:
        with (
            tc.tile_pool(name="consts", bufs=1) as consts,
            tc.tile_pool(name="big", bufs=1) as big,
        ):
            # ---- constants ----
            ident = consts.tile([128, 128], F32)
            make_identity(nc, ident)
            ones_r = consts.tile([1, 128], BF16)
            nc.vector.memset(ones_r, 1.0)
            ones_12 = consts.tile([128, 12], F32)
            nc.vector.memset(ones_12, 1.0)
            eps_t = consts.tile([128, 1], F32)
            nc.vector.memset(eps_t, EPS)

            bqc = consts.tile([128, FT], F32)
            bkc = consts.tile([128, FT], F32)
            for f in range(FT):
                nc.sync.dma_start(
                    out=bqc[:, f : f + 1],
                    in_=bq_d[0:1, f * 128 : (f + 1) * 128].rearrange("a b -> b a"),
                )
                nc.sync.dma_start(
                    out=bkc[:, f : f + 1],
                    in_=bk_d[0:1, f * 128 : (f + 1) * 128].rearrange("a b -> b a"),
                )
            bv_f = consts.tile([1, HID], F32)
            nc.sync.dma_start(out=bv_f, in_=bv_d)
            bd_f = consts.tile([1, HID], F32)
            nc.sync.dma_start(out=bd_f, in_=bd_d)
            bv_row = consts.tile([1, HID], BF16)
            nc.vector.tensor_copy(out=bv_row, in_=bv_f)
            bd_row = consts.tile([1, HID], BF16)
            nc.vector.tensor_copy(out=bd_row, in_=bd_f)

            mask_t = consts.tile([128, ST], F32)
            for t in range(ST):
                nc.sync.dma_start(
                    out=mask_t[:, t : t + 1], in_=mask_d[t * 128 : (t + 1) * 128, :]
                )
            emask = consts.tile([128, ST], F32)
            nc.scalar.activation(out=emask, in_=mask_t, func=AF.Exp)

            # broadcast ln gamma/beta to all 128 partitions (stride-0 DMA)
            g_bc = consts.tile([128, HID], F32)
            b_bc = consts.tile([128, HID], F32)
            nc.sync.dma_start(out=g_bc, in_=_bcast_part(lng_d))
            nc.sync.dma_start(out=b_bc, in_=_bcast_part(lnb_d))

            # ---- persistent activation buffers ----
            qT = [big.tile([128, S], BF16, name=f"qT{f}") for f in range(FT)]
            kT = [big.tile([128, S], BF16, name=f"kT{f}") for f in range(FT)]
            vb = [big.tile([128, H, HD + 1], BF16, name=f"vb{t}") for t in range(ST)]
            # wd stored per-head ([64, 768] at partition base 0) so the dense
            # per-head K=64 matmuls have base-aligned lhsT/rhs
            dw_bf = [big.tile([HD, HID], BF16, name=f"dwbf{h}") for h in range(H)]

            # ---- projections ----
            def project_chunk(x_d, xT_c, ps_tp, xn_pool, chunk):
                """DMA 512 rows of x, transpose into xT_c [128, FT, 512] f32."""
                for ss in range(4):
                    x_nat = xn_pool.tile([128, HID], F32, name="x_nat")
                    st = chunk * 4 + ss
                    nc.sync.dma_start(
                        out=x_nat, in_=x_d[st * 128 : (st + 1) * 128, :]
                    )
                    for f in range(FT):
                        tp_ps = ps_tp.tile([128, 128], F32, name="tp_ps")
                        nc.tensor.transpose(
                            tp_ps, x_nat[:, f * 128 : (f + 1) * 128], ident
                        )
                        nc.vector.tensor_copy(
                            out=xT_c[:, f, ss * 128 : (ss + 1) * 128], in_=tp_ps
                        )

            with (
                tc.tile_pool(name="wq_pool", bufs=1) as wq_pool,
                tc.tile_pool(name="xn", bufs=3) as xn_pool,
                tc.tile_pool(name="xT", bufs=2) as xT_pool,
                tc.tile_pool(name="ps_tp", bufs=4, space="PSUM") as ps_tp,
                tc.tile_pool(name="ps_pj", bufs=2, space="PSUM") as ps_pj,
            ):
                wq_b = [
                    wq_pool.tile([128, HID], BF16, name=f"wq{f}") for f in range(FT)
                ]
                for f in range(FT):
                    wtmp = xn_pool.tile([128, HID], F32, name="wtmp")
                    nc.sync.dma_start(out=wtmp, in_=wq_d[f * 128 : (f + 1) * 128, :])
                    nc.vector.tensor_copy(out=wq_b[f], in_=wtmp)
                # load wd (fp32) per head and convert to bf16
                for h in range(H):
                    wd_t = xn_pool.tile([HD, HID], F32, name="wd_t")
                    nc.sync.dma_start(out=wd_t, in_=wd_d[h * HD : (h + 1) * HD, :])
                    nc.vector.tensor_copy(out=dw_bf[h], in_=wd_t)

                for chunk in range(QT):
                    xT_c = xT_pool.tile([128, FT, 512], BF16, name="xT_q")
                    project_chunk(xq_d, xT_c, ps_tp, xn_pool, chunk)
                    for fo in range(FT):
                        pj = ps_pj.tile([128, 512], F32, name="pj")
                        for kf in range(FT):
                            nc.tensor.matmul(
                                pj,
                                wq_b[kf][:, fo * 128 : (fo + 1) * 128],
                                xT_c[:, kf, :],
                                start=(kf == 0),
                                stop=(kf == FT - 1),
                            )
                        nc.vector.tensor_scalar_add(
                            out=qT[fo][:, chunk * 512 : (chunk + 1) * 512],
                            in0=pj,
                            scalar1=bqc[:, fo : fo + 1],
                        )

            with (
                tc.tile_pool(name="wkv_pool", bufs=1) as wkv_pool,
                tc.tile_pool(name="xn2", bufs=3) as xn2_pool,
                tc.tile_pool(name="xT2", bufs=2) as xT2_pool,
                tc.tile_pool(name="ps_tp2", bufs=2, space="PSUM") as ps_tp2,
                tc.tile_pool(name="ps_pj2", bufs=2, space="PSUM") as ps_pj2,
                tc.tile_pool(name="ps_v", bufs=2, space="PSUM") as ps_v,
            ):
                wk_b = [
                    wkv_pool.tile([128, HID], BF16, name=f"wk{f}") for f in range(FT)
                ]
                wv_b = [
                    wkv_pool.tile([128, HID], BF16, name=f"wv{f}") for f in range(FT)
                ]
                for f in range(FT):
                    wtmp = xn2_pool.tile([128, HID], F32, name="wtmp2")
                    nc.sync.dma_start(out=wtmp, in_=wk_d[f * 128 : (f + 1) * 128, :])
                    nc.vector.tensor_copy(out=wk_b[f], in_=wtmp)
                    wtmp = xn2_pool.tile([128, HID], F32, name="wtmp2")
                    nc.sync.dma_start(out=wtmp, in_=wv_d[f * 128 : (f + 1) * 128, :])
                    nc.vector.tensor_copy(out=wv_b[f], in_=wtmp)

                for chunk in range(QT):
                    xT_c = xT2_pool.tile([128, FT, 512], BF16, name="xT_kv")
                    project_chunk(xkv_d, xT_c, ps_tp2, xn2_pool, chunk)
                    # kT
                    for fo in range(FT):
                        pj = ps_pj2.tile([128, 512], F32, name="pj2")
                        for kf in range(FT):
                            nc.tensor.matmul(
                                pj,
                                wk_b[kf][:, fo * 128 : (fo + 1) * 128],
                                xT_c[:, kf, :],
                                start=(kf == 0),
                                stop=(kf == FT - 1),
                            )
                        nc.vector.tensor_scalar_add(
                            out=kT[fo][:, chunk * 512 : (chunk + 1) * 512],
                            in0=pj,
                            scalar1=bkc[:, fo : fo + 1],
                        )
                    # v (natural layout, rows scaled by exp(mask), + denom col)
                    for ss in range(4):
                        st = chunk * 4 + ss
                        vp = ps_v.tile([128, NH, 512], F32, name="vp")
                        for nh in range(NH):
                            for kf in range(FT):
                                nc.tensor.matmul(
                                    vp[:, nh, 0:NW],
                                    xT_c[:, kf, ss * 128 : (ss + 1) * 128],
                                    wv_b[kf][:, nh * NW : (nh + 1) * NW],
                                    start=(kf == 0),
                                    stop=False,
                                )
                            nc.tensor.matmul(
                                vp[:, nh, 0:NW],
                                ones_r,
                                bv_row[0:1, nh * NW : (nh + 1) * NW],
                                start=False,
                                stop=True,
                            )
                        emcol = emask[:, st : st + 1]
                        for nh in range(NH):
                            nc.vector.tensor_scalar_mul(
                                out=vb[st][:, nh * 6 : (nh + 1) * 6, 0:HD],
                                in0=vp[:, nh, 0:NW].rearrange(
                                    "p (a d) -> p a d", a=6
                                ),
                                scalar1=emcol,
                            )
                        nc.vector.tensor_scalar_mul(
                            out=vb[st][:, :, HD : HD + 1].rearrange(
                                "p a c -> p (a c)"
                            ),
                            in0=ones_12,
                            scalar1=emcol,
                        )

            # ---- attention + dense + layernorm, per 512-wide q chunk ----
            with (
                tc.tile_pool(name="ctx_pool", bufs=2) as ctx_pool,
                tc.tile_pool(name="ctxu_pool", bufs=1) as ctxu_pool,
                tc.tile_pool(name="dram_pool", bufs=2, space="DRAM") as dram_pool,
                tc.tile_pool(name="exp_pool", bufs=4) as exp_pool,
                tc.tile_pool(name="rec_pool", bufs=2) as rec_pool,
                tc.tile_pool(name="res_pool", bufs=3) as res_pool,
                tc.tile_pool(name="hpre_pool", bufs=2) as hpre_pool,
                tc.tile_pool(name="st_pool", bufs=4) as st_pool,
                tc.tile_pool(name="ps_sc", bufs=2, space="PSUM") as ps_sc,
                tc.tile_pool(name="ps_ctx", bufs=2, space="PSUM") as ps_ctx,
                tc.tile_pool(name="ps_h", bufs=2, space="PSUM") as ps_h,
            ):
                def make_dense_steps(qt, ctx_t):
                    """Dense + residual + LN for chunk qt as 9 deferred steps,
                    emitted between the next chunk's attention heads so the
                    in-order PE has fill work while ACT computes exps."""
                    state = {}

                    def group_step(ss, nh):
                        def run():
                            if "mvq" not in state:
                                state["mvq"] = st_pool.tile(
                                    [128, 4, 2], F32, name="mvq"
                                )
                                state["hp"] = {}
                            st = qt * 4 + ss
                            ssl = slice(ss * 128, (ss + 1) * 128)
                            if ss not in state["hp"]:
                                state["hp"][ss] = hpre_pool.tile(
                                    [128, HID], F32, name=f"hp{ss}"
                                )
                            hp = state["hp"][ss]
                            h_ps = ps_h.tile([128, 512], F32, name="h_ps")
                            for hh in range(H):
                                nc.tensor.matmul(
                                    h_ps[:, 0:NW],
                                    ctx_t[hh][:, ssl],
                                    dw_bf[hh][:, nh * NW : (nh + 1) * NW],
                                    start=(hh == 0),
                                    stop=False,
                                )
                            nc.tensor.matmul(
                                h_ps[:, 0:NW],
                                ones_r,
                                bd_row[0:1, nh * NW : (nh + 1) * NW],
                                start=False,
                                stop=True,
                            )
                            x_res = res_pool.tile([128, NW], F32, name="x_res")
                            nc.sync.dma_start(
                                out=x_res,
                                in_=xkv_d[
                                    st * 128 : (st + 1) * 128,
                                    nh * NW : (nh + 1) * NW,
                                ],
                            )
                            nc.vector.tensor_add(
                                out=hp[:, nh * NW : (nh + 1) * NW],
                                in0=h_ps[:, 0:NW],
                                in1=x_res,
                            )
                            if nh == NH - 1:
                                stats = st_pool.tile([128, 3, 6], F32, name="stats")
                                for sg in range(3):
                                    nc.vector.bn_stats(
                                        out=stats[:, sg, :],
                                        in_=hp[:, sg * 256 : (sg + 1) * 256],
                                    )
                                nc.vector.bn_aggr(
                                    out=state["mvq"][:, ss, :], in_=stats
                                )

                        return run

                    def tail():
                        mvq = state["mvq"]
                        # rstd = exp(-0.5*ln(var+eps)), batched over the 4
                        # subtiles (2 ACT table switches per chunk)
                        lnv = st_pool.tile([128, 4], F32, name="lnv")
                        nc.scalar.activation(
                            out=lnv, in_=mvq[:, :, 1], func=AF.Ln,
                            bias=eps_t, scale=1.0,
                        )
                        rstd4 = st_pool.tile([128, 4], F32, name="rstd4")
                        nc.scalar.activation(
                            out=rstd4, in_=lnv, func=AF.Exp, scale=-0.5
                        )
                        for ss in range(4):
                            st = qt * 4 + ss
                            hp = state["hp"][ss]
                            hn = hpre_pool.tile([128, HID], F32, name="hn")
                            nc.vector.tensor_scalar(
                                out=hn,
                                in0=hp,
                                scalar1=mvq[:, ss, 0:1],
                                scalar2=rstd4[:, ss : ss + 1],
                                op0=mybir.AluOpType.subtract,
                                op1=mybir.AluOpType.mult,
                            )
                            nc.vector.tensor_mul(hn, hn, g_bc)
                            nc.vector.tensor_add(hn, hn, b_bc)
                            nc.sync.dma_start(
                                out=out_d[st * 128 : (st + 1) * 128, :], in_=hn
                            )

                    return [group_step(ss, nh) for ss in range(4) for nh in range(NH)] + [tail]

                pending = []
                for qt in range(QT):
                    ctx_t = [
                        ctx_pool.tile([HD, 512], BF16, name=f"ctx{h}")
                        for h in range(H)
                    ]
                    ctxu = [
                        ctxu_pool.tile([HD, 512], BF16, name=f"ctxu{h}")
                        for h in range(H)
                    ]
                    den_all = rec_pool.tile([H, 512], F32, name="den_all")
                    qsl = slice(qt * 512, (qt + 1) * 512)
                    for h in range(H):
                        ft, po = h // 2, (h % 2) * 64
                        ctx_ps = ps_ctx.tile([HD + 1, 512], F32, name="ctx_ps")
                        # software-pipelined: scores(g) are emitted before
                        # ctx(g-1) so the in-order PE streams scores while
                        # ACT computes exp(g-1) — no per-group PE stall
                        exps = []
                        for g in range(8):
                            sc_ps = ps_sc.tile([128, 2, 512], F32, name="sc_ps")
                            for j in range(2):
                                kc = g * 2 + j
                                nc.tensor.matmul(
                                    sc_ps[:, j, :],
                                    kT[ft][po : po + HD, kc * 128 : (kc + 1) * 128],
                                    qT[ft][po : po + HD, qsl],
                                    start=True,
                                    stop=True,
                                )
                            exp_g = exp_pool.tile([128, 2, 512], BF16, name="exp_g")
                            nc.scalar.activation(
                                out=exp_g, in_=sc_ps, func=AF.Exp, scale=0.125
                            )
                            exps.append(exp_g)
                            if g == 4 and pending:
                                pending.pop(0)()  # fill PE while ACT works
                            if g > 0:
                                for j in range(2):
                                    kc = (g - 1) * 2 + j
                                    nc.tensor.matmul(
                                        ctx_ps,
                                        vb[kc][:, h, :],
                                        exps[g - 1][:, j, :],
                                        start=(g == 1 and j == 0),
                                        stop=False,
                                    )
                        for j in range(2):
                            kc = 7 * 2 + j
                            nc.tensor.matmul(
                                ctx_ps,
                                vb[kc][:, h, :],
                                exps[7][:, j, :],
                                start=False,
                                stop=(j == 1),
                            )
                        # evict unnormalized ctx + denominator; normalization
                        # is batched per chunk (below), off the head loop
                        dtmp = rec_pool.tile([HD + 1, 512], F32, name="dtmp")
                        nc.vector.tensor_copy(
                            out=dtmp[HD : HD + 1, :], in_=ctx_ps[HD : HD + 1, :]
                        )
                        nc.sync.dma_start(
                            out=den_all[h : h + 1, :], in_=dtmp[HD : HD + 1, :]
                        )
                        nc.vector.tensor_copy(out=ctxu[h], in_=ctx_ps[0:HD, :])
                    # one iterative-divide pass for all 12 heads' denominators
                    rec_all = rec_pool.tile([H, 512], F32, name="rec_all")
                    nc.vector.reciprocal(rec_all, den_all)
                    # partition-broadcast needs a DRAM source: bounce the
                    # reciprocal rows out, then stride-0 DMA back per head
                    rec_d = dram_pool.tile([H, 512], F32, name="rec_d")
                    nc.sync.dma_start(out=rec_d, in_=rec_all)
                    for h in range(H):
                        bc_sb = rec_pool.tile([HD, 512], F32, name="bc_sb")
                        nc.sync.dma_start(
                            out=bc_sb,
                            in_=rec_d[h : h + 1, :].to_broadcast((HD, 512)),
                        )
                        nc.vector.tensor_mul(
                            out=ctx_t[h], in0=ctxu[h], in1=bc_sb
                        )
                        if pending and h % 2 == 0:
                            pending.pop(0)()

                    pending = make_dense_steps(qt, ctx_t)
                for step in pending:
                    step()

    nc.compile()
    return nc


_NC = None


def _get_nc():
    global _NC
    if _NC is None:
        _NC = build_nc()
    return _NC


def _prepare(
    input_tensor1, attention_mask1, input_tensor2, attention_mask2,
    q1_w, q1_b, k1_w, k1_b, v1_w, v1_b,
    q2_w, q2_b, k2_w, k2_b, v2_w, v2_b,
    d1_w, d1_b, d2_w, d2_b, ln1_g, ln1_b, ln2_g, ln2_b,
):
    f = lambda a: np.ascontiguousarray(np.asarray(a), dtype=np.float32)
    x1, x2 = f(input_tensor1), f(input_tensor2)
    m1 = f(attention_mask1).reshape(B, S, 1)
    m2 = f(attention_mask2).reshape(B, S, 1)
    row = lambda a: f(a).reshape(1, HID)

    in_maps = []
    for b in range(B):
        # stream1: ctx1 = attend(q2, k1, v1, mask1); out h1[b]
        in_maps.append({
            "xq": x2[b], "xkv": x1[b],
            "wq": f(q2_w), "wk": f(k1_w), "wv": f(v1_w), "wd": f(d1_w),
            "bq": row(q2_b), "bk": row(k1_b), "bv": row(v1_b), "bd": row(d1_b),
            "mask": m1[b], "lng": row(ln1_g), "lnb": row(ln1_b),
        })
    for b in range(B):
        # stream2: ctx2 = attend(q1, k2, v2, mask2); out h2[b]
        in_maps.append({
            "xq": x1[b], "xkv": x2[b],
            "wq": f(q1_w), "wk": f(k2_w), "wv": f(v2_w), "wd": f(d2_w),
            "bq": row(q1_b), "bk": row(k2_b), "bv": row(v2_b), "bd": row(d2_b),
            "mask": m2[b], "lng": row(ln2_g), "lnb": row(ln2_b),
        })

    return in_maps


def _run(in_maps, **kwargs):
    nc = _get_nc()
    res = bass_utils.run_bass_kernel_spmd(
        nc, in_maps, core_ids=list(range(8)), **kwargs
    )
    h1 = np.stack([res.results[b]["out"] for b in range(B)])
    h2 = np.stack([res.results[B + b]["out"] for b in range(B)])
    return (h1, h2), res


def kernel(**inputs):
    (h1, h2), _ = _run(_prepare(**inputs))
    return h1, h2


# revision 61
# speedup vs baseline: 1.1796x; 1.0375x over previous
"""BertBiAttention Trainium2 kernel.

Cross-attention between two streams (B=4, S=2048, HID=768, H=12 heads).
Sharding: 8 cores = (stream s in {1,2}) x (batch b in {0..3}). Each core
computes one stream's full output for one batch element:
    h_s[b] = LayerNorm( attend(q_other, k_own, v_own, mask_own) @ wd + bd + x_own )
No collectives needed; the host stacks per-core outputs.

On-chip layouts (per core, all matmuls bf16 with fp32 PSUM accumulation):
  qT, kT  [768, 2048] bf16  (feature-major; head h at partition rows h*64..)
  v       16 x [128, 12, 65] bf16  (per head: [v*exp(mask) | exp(mask)])
  scoresT [krows, q] in PSUM -> exp(s/8) on ACT -> bf16 (sc->exp->ctx
          software-pipelined; dense steps of the previous q-chunk are
          interleaved between heads as PE fill work)
  ctx     lhsT=[v|em] matmuls accumulate [ctx | denom]; denominators of all
          12 heads batched into one DVE reciprocal, broadcast back via a
          DRAM-bounce stride-0 DMA, normalized with one multiply per head
  dense   per-head K=64 matmuls (+bd via K=1 ones matmul) + residual;
          LayerNorm rstd = exp(-0.5*ln(var+eps)) keeps ACT on one table set.
"""

import numpy as np

import concourse.bass as bass
import concourse.mybir as mybir
import concourse.tile as tile
from concourse import bacc, bass_utils
from concourse.masks import make_identity

B, S, HID, H, HD = 4, 2048, 768, 12, 64
FT = HID // 128   # 6 feature tiles
ST = S // 128     # 16 seq tiles
QT = S // 512     # 4 q chunks
NH = 2            # 768-wide outputs split into 2 x 384
NW = 384
EPS = 1e-12

F32 = mybir.dt.float32
F32R = mybir.dt.float32r
BF16 = mybir.dt.bfloat16
AF = mybir.ActivationFunctionType


def _bcast_part(ap, p=128):
    """DRAM row [1, N] -> partition-broadcast AP [p, N] (stride-0 partition)."""
    return bass.AP(tensor=ap.tensor, offset=ap.offset, ap=[[0, p], ap.ap[-1]])


def _setup_act_tables():
    """Point the compiler at an act_info.json whose first set covers both
    exp and ln (natural_log_exp_and_others), so the kernel's Exp and Ln
    activations share one ACT table set instead of reloading (~1.3us) on
    every switch."""
    import json
    import os
    import tempfile
    from pathlib import Path

    if os.environ.get("BASS_ACT_ROOT_JSON_PATH"):
        return
    try:
        from neuronxcc.driver.Job import Job
        from neuronxcc.driver.jobs.support.FindActInfo import findActInfoFile

        src = Path(findActInfoFile(Job.getPackageDir(), "gen3"))
        d = json.loads(src.read_text())
        sets = d["act_func_sets"]
        pref = [s for s in sets if s["name"] == "natural_log_exp_and_others"]
        rest = [s for s in sets if s["name"] != "natural_log_exp_and_others"]
        if not pref:
            return
        d["act_func_sets"] = pref + rest
        dst = Path(tempfile.mkdtemp(prefix="act_tables_"))
        for f in src.parent.iterdir():
            if f.name != src.name and f.is_file():
                os.symlink(f, dst / f.name)
        (dst / src.name).write_text(json.dumps(d))
        os.environ["BASS_ACT_ROOT_JSON_PATH"] = str(dst / src.name)
    except Exception:
        pass  # default tables still work, just slower


def build_nc():
    # _setup_act_tables()  # crashes the exec unit via this compile path
    nc = bacc.Bacc("TRN2", target_bir_lowering=False, debug=False, num_devices=8)

    xq_d = nc.dram_tensor("xq", [S, HID], F32, kind="ExternalInput").ap()
    xkv_d = nc.dram_tensor("xkv", [S, HID], F32, kind="ExternalInput").ap()
    wq_d = nc.dram_tensor("wq", [HID, HID], F32, kind="ExternalInput").ap()
    wk_d = nc.dram_tensor("wk", [HID, HID], F32, kind="ExternalInput").ap()
    wv_d = nc.dram_tensor("wv", [HID, HID], F32, kind="ExternalInput").ap()
    wd_d = nc.dram_tensor("wd", [HID, HID], F32, kind="ExternalInput").ap()
    bq_d = nc.dram_tensor("bq", [1, HID], F32, kind="ExternalInput").ap()
    bk_d = nc.dram_tensor("bk", [1, HID], F32, kind="ExternalInput").ap()
    bv_d = nc.dram_tensor("bv", [1, HID], F32, kind="ExternalInput").ap()
    bd_d = nc.dram_tensor("bd", [1, HID], F32, kind="ExternalInput").ap()
    mask_d = nc.dram_tensor("mask", [S, 1], F32, kind="ExternalInput").ap()
    lng_d = nc.dram_tensor("lng", [1, HID], F32, kind="ExternalInput").ap()
    lnb_d = nc.dram_tensor("lnb", [1, HID], F32, kind="ExternalInput").ap()
    out_d = nc.dram_tensor("out", [S, HID], F32, kind="ExternalOutput").ap()

    with tile.TileContext(nc) as tc:
        with (
            tc.tile_pool(name="consts", bufs=1) as consts,
            tc.tile_pool(name="big", bufs=1) as big,
        ):
            # ---- constants ----
            ident = consts.tile([128, 128], F32)
            make_identity(nc, ident)
            ones_r = consts.tile([1, 128], BF16)
            nc.vector.memset(ones_r, 1.0)
            ones_12 = consts.tile([128, 12], F32)
            nc.vector.memset(ones_12, 1.0)
            eps_t = consts.tile([128, 1], F32)
            nc.vector.memset(eps_t, EPS)

            bqc = consts.tile([128, FT], F32)
            bkc = consts.tile([128, FT], F32)
            for f in range(FT):
                nc.sync.dma_start(
                    out=bqc[:, f : f + 1],
                    in_=bq_d[0:1, f * 128 : (f + 1) * 128].rearrange("a b -> b a"),
                )
                nc.sync.dma_start(
                    out=bkc[:, f : f + 1],
                    in_=bk_d[0:1, f * 128 : (f + 1) * 128].rearrange("a b -> b a"),
                )
            bv_f = consts.tile([1, HID], F32)
            nc.sync.dma_start(out=bv_f, in_=bv_d)
            bd_f = consts.tile([1, HID], F32)
            nc.sync.dma_start(out=bd_f, in_=bd_d)
            bv_row = consts.tile([1, HID], BF16)
            nc.vector.tensor_copy(out=bv_row, in_=bv_f)
            bd_row = consts.tile([1, HID], BF16)
            nc.vector.tensor_copy(out=bd_row, in_=bd_f)

            mask_t = consts.tile([128, ST], F32)
            for t in range(ST):
                nc.sync.dma_start(
                    out=mask_t[:, t : t + 1], in_=mask_d[t * 128 : (t + 1) * 128, :]
                )
            emask = consts.tile([128, ST], F32)
            nc.scalar.activation(out=emask, in_=mask_t, func=AF.Exp)

            # broadcast ln gamma/beta to all 128 partitions (stride-0 DMA)
            g_bc = consts.tile([128, HID], F32)
            b_bc = consts.tile([128, HID], F32)
            nc.sync.dma_start(out=g_bc, in_=_bcast_part(lng_d))
            nc.sync.dma_start(out=b_bc, in_=_bcast_part(lnb_d))

            # ---- persistent activation buffers ----
            qT = [big.tile([128, S], BF16, name=f"qT{f}") for f in range(FT)]
            kT = [big.tile([128, S], BF16, name=f"kT{f}") for f in range(FT)]
            vb = [big.tile([128, H, HD + 1], BF16, name=f"vb{t}") for t in range(ST)]
            # wd stored per-head ([64, 768] at partition base 0) so the dense
            # per-head K=64 matmuls have base-aligned lhsT/rhs
            dw_bf = [big.tile([HD, HID], BF16, name=f"dwbf{h}") for h in range(H)]

            # ---- projections ----
            def project_chunk(x_d, xT_c, ps_tp, xn_pool, chunk):
                """DMA 512 rows of x, transpose into xT_c [128, FT, 512] f32."""
                for ss in range(4):
                    x_nat = xn_pool.tile([128, HID], F32, name="x_nat")
                    st = chunk * 4 + ss
                    nc.sync.dma_start(
                        out=x_nat, in_=x_d[st * 128 : (st + 1) * 128, :]
                    )
                    for f in range(FT):
                        tp_ps = ps_tp.tile([128, 128], F32, name="tp_ps")
                        nc.tensor.transpose(
                            tp_ps, x_nat[:, f * 128 : (f + 1) * 128], ident
                        )
                        nc.vector.tensor_copy(
                            out=xT_c[:, f, ss * 128 : (ss + 1) * 128], in_=tp_ps
                        )

            with (
                tc.tile_pool(name="wkv_pool", bufs=1) as wkv_pool,
                tc.tile_pool(name="xn2", bufs=3) as xn2_pool,
                tc.tile_pool(name="xT2", bufs=2) as xT2_pool,
                tc.tile_pool(name="ps_tp2", bufs=2, space="PSUM") as ps_tp2,
                tc.tile_pool(name="ps_pj2", bufs=2, space="PSUM") as ps_pj2,
                tc.tile_pool(name="ps_v", bufs=2, space="PSUM") as ps_v,
            ):
                wk_b = [
                    wkv_pool.tile([128, HID], BF16, name=f"wk{f}") for f in range(FT)
                ]
                wv_b = [
                    wkv_pool.tile([128, HID], BF16, name=f"wv{f}") for f in range(FT)
                ]
                for f in range(FT):
                    wtmp = xn2_pool.tile([128, HID], F32, name="wtmp2")
                    nc.sync.dma_start(out=wtmp, in_=wk_d[f * 128 : (f + 1) * 128, :])
                    nc.vector.tensor_copy(out=wk_b[f], in_=wtmp)
                    wtmp = xn2_pool.tile([128, HID], F32, name="wtmp2")
                    nc.sync.dma_start(out=wtmp, in_=wv_d[f * 128 : (f + 1) * 128, :])
                    nc.vector.tensor_copy(out=wv_b[f], in_=wtmp)

                for chunk in range(QT):
                    xT_c = xT2_pool.tile([128, FT, 512], BF16, name="xT_kv")
                    project_chunk(xkv_d, xT_c, ps_tp2, xn2_pool, chunk)
                    # kT
                    for fo in range(FT):
                        pj = ps_pj2.tile([128, 512], F32, name="pj2")
                        for kf in range(FT):
                            nc.tensor.matmul(
                                pj,
                                wk_b[kf][:, fo * 128 : (fo + 1) * 128],
                                xT_c[:, kf, :],
                                start=(kf == 0),
                                stop=(kf == FT - 1),
                            )
                        nc.vector.tensor_scalar_add(
                            out=kT[fo][:, chunk * 512 : (chunk + 1) * 512],
                            in0=pj,
                            scalar1=bkc[:, fo : fo + 1],
                        )
                    # v (natural layout, rows scaled by exp(mask), + denom col)
                    for ss in range(4):
                        st = chunk * 4 + ss
                        vp = ps_v.tile([128, NH, 512], F32, name="vp")
                        for nh in range(NH):
                            for kf in range(FT):
                                nc.tensor.matmul(
                                    vp[:, nh, 0:NW],
                                    xT_c[:, kf, ss * 128 : (ss + 1) * 128],
                                    wv_b[kf][:, nh * NW : (nh + 1) * NW],
                                    start=(kf == 0),
                                    stop=False,
                                )
                            nc.tensor.matmul(
                                vp[:, nh, 0:NW],
                                ones_r,
                                bv_row[0:1, nh * NW : (nh + 1) * NW],
                                start=False,
                                stop=True,
                            )
                        emcol = emask[:, st : st + 1]
                        for nh in range(NH):
                            nc.vector.tensor_scalar_mul(
                                out=vb[st][:, nh * 6 : (nh + 1) * 6, 0:HD],
                                in0=vp[:, nh, 0:NW].rearrange(
                                    "p (a d) -> p a d", a=6
                                ),
                                scalar1=emcol,
                            )
                        nc.vector.tensor_scalar_mul(
                            out=vb[st][:, :, HD : HD + 1].rearrange(
                                "p a c -> p (a c)"
                            ),
                            in0=ones_12,
                            scalar1=emcol,
                        )

            # ---- attention + dense + layernorm, per 512-wide q chunk ----
            with (
                tc.tile_pool(name="wq_pool", bufs=1) as wq_pool,
                tc.tile_pool(name="xnq", bufs=2) as xnq_pool,
                tc.tile_pool(name="xTq", bufs=1) as xTq_pool,
                tc.tile_pool(name="ps_tp", bufs=1, space="PSUM") as ps_tp,
                tc.tile_pool(name="ps_pj", bufs=1, space="PSUM") as ps_pj,
                tc.tile_pool(name="ctx_pool", bufs=2) as ctx_pool,
                tc.tile_pool(name="dram_pool", bufs=2, space="DRAM") as dram_pool,
                tc.tile_pool(name="exp_pool", bufs=4) as exp_pool,
                tc.tile_pool(name="rec_pool", bufs=2) as rec_pool,
                tc.tile_pool(name="res_pool", bufs=3) as res_pool,
                tc.tile_pool(name="hpre_pool", bufs=1) as hpre_pool,
                tc.tile_pool(name="st_pool", bufs=4) as st_pool,
                tc.tile_pool(name="ps_sc", bufs=2, space="PSUM") as ps_sc,
                tc.tile_pool(name="ps_ctx", bufs=1, space="PSUM") as ps_ctx,
                tc.tile_pool(name="ps_h", bufs=1, space="PSUM") as ps_h,
            ):
                wq_b = [
                    wq_pool.tile([128, HID], BF16, name=f"wq{f}") for f in range(FT)
                ]
                for f in range(FT):
                    wtmp = xnq_pool.tile([128, HID], F32, name="x_nat")
                    nc.sync.dma_start(out=wtmp, in_=wq_d[f * 128 : (f + 1) * 128, :])
                    nc.vector.tensor_copy(out=wq_b[f], in_=wtmp)
                for h in range(H):
                    wd_t = xnq_pool.tile([HD, HID], F32, name="wd_t")
                    nc.sync.dma_start(out=wd_t, in_=wd_d[h * HD : (h + 1) * HD, :])
                    nc.vector.tensor_copy(out=dw_bf[h], in_=wd_t)

                def q_proj_mm(chunk, xT_c, fo_range):
                    for fo in fo_range:
                        pj = ps_pj.tile([128, 512], F32, name="pj")
                        for kf in range(FT):
                            nc.tensor.matmul(
                                pj,
                                wq_b[kf][:, fo * 128 : (fo + 1) * 128],
                                xT_c[:, kf, :],
                                start=(kf == 0),
                                stop=(kf == FT - 1),
                            )
                        nc.vector.tensor_scalar_add(
                            out=qT[fo][:, chunk * 512 : (chunk + 1) * 512],
                            in0=pj,
                            scalar1=bqc[:, fo : fo + 1],
                        )

                def q_transpose_ss(xT_c, chunk, ss):
                    x_nat = xnq_pool.tile([128, HID], F32, name="x_nat")
                    st = chunk * 4 + ss
                    nc.sync.dma_start(
                        out=x_nat, in_=xq_d[st * 128 : (st + 1) * 128, :]
                    )
                    for f in range(FT):
                        tp_ps = ps_tp.tile([128, 128], F32, name="tp_q")
                        nc.tensor.transpose(
                            tp_ps, x_nat[:, f * 128 : (f + 1) * 128], ident
                        )
                        nc.vector.tensor_copy(
                            out=xT_c[:, f, ss * 128 : (ss + 1) * 128], in_=tp_ps
                        )

                def q_proj_steps(chunk):
                    state = {}

                    def tstep(ss_pair):
                        def run():
                            if "xT" not in state:
                                state["xT"] = xTq_pool.tile(
                                    [128, FT, 512], BF16, name="xT_q"
                                )
                            for ss in ss_pair:
                                q_transpose_ss(state["xT"], chunk, ss)

                        return run

                    def mstep(fo_range):
                        return lambda: q_proj_mm(chunk, state["xT"], fo_range)

                    return [
                        tstep((0, 1)),
                        tstep((2, 3)),
                        mstep(range(0, 2)),
                        mstep(range(2, 4)),
                        mstep(range(4, 6)),
                    ]

                def make_dense_steps(qt, ctx_t):
                    """Dense + residual + LN for chunk qt as 9 deferred steps,
                    emitted between the next chunk's attention heads so the
                    in-order PE has fill work while ACT computes exps."""
                    state = {}

                    def group_step(ss, nh):
                        def run():
                            if "mvq" not in state:
                                state["mvq"] = st_pool.tile(
                                    [128, 4, 2], F32, name="mvq"
                                )
                                state["hp"] = {}
                            st = qt * 4 + ss
                            ssl = slice(ss * 128, (ss + 1) * 128)
                            if ss not in state["hp"]:
                                state["hp"][ss] = hpre_pool.tile(
                                    [128, HID], F32, name=f"hp{ss}"
                                )
                            hp = state["hp"][ss]
                            h_ps = ps_h.tile([128, 512], F32, name="h_ps")
                            for hh in range(H):
                                nc.tensor.matmul(
                                    h_ps[:, 0:NW],
                                    ctx_t[hh][:, ssl],
                                    dw_bf[hh][:, nh * NW : (nh + 1) * NW],
                                    start=(hh == 0),
                                    stop=False,
                                )
                            nc.tensor.matmul(
                                h_ps[:, 0:NW],
                                ones_r,
                                bd_row[0:1, nh * NW : (nh + 1) * NW],
                                start=False,
                                stop=True,
                            )
                            x_res = res_pool.tile([128, NW], F32, name="x_res")
                            nc.sync.dma_start(
                                out=x_res,
                                in_=xkv_d[
                                    st * 128 : (st + 1) * 128,
                                    nh * NW : (nh + 1) * NW,
                                ],
                            )
                            nc.vector.tensor_add(
                                out=hp[:, nh * NW : (nh + 1) * NW],
                                in0=h_ps[:, 0:NW],
                                in1=x_res,
                            )
                            if nh == NH - 1:
                                stats = st_pool.tile([128, 3, 6], F32, name="stats")
                                for sg in range(3):
                                    nc.vector.bn_stats(
                                        out=stats[:, sg, :],
                                        in_=hp[:, sg * 256 : (sg + 1) * 256],
                                    )
                                nc.vector.bn_aggr(
                                    out=state["mvq"][:, ss, :], in_=stats
                                )

                        return run

                    def tail():
                        mvq = state["mvq"]
                        # rstd = exp(-0.5*ln(var+eps)), batched over the 4
                        # subtiles (2 ACT table switches per chunk)
                        lnv = st_pool.tile([128, 4], F32, name="lnv")
                        nc.scalar.activation(
                            out=lnv, in_=mvq[:, :, 1], func=AF.Ln,
                            bias=eps_t, scale=1.0,
                        )
                        rstd4 = st_pool.tile([128, 4], F32, name="rstd4")
                        nc.scalar.activation(
                            out=rstd4, in_=lnv, func=AF.Exp, scale=-0.5
                        )
                        for ss in range(4):
                            st = qt * 4 + ss
                            hp = state["hp"][ss]
                            hn = hpre_pool.tile([128, HID], F32, name="hn")
                            nc.vector.tensor_scalar(
                                out=hn,
                                in0=hp,
                                scalar1=mvq[:, ss, 0:1],
                                scalar2=rstd4[:, ss : ss + 1],
                                op0=mybir.AluOpType.subtract,
                                op1=mybir.AluOpType.mult,
                            )
                            nc.vector.tensor_mul(hn, hn, g_bc)
                            nc.vector.tensor_add(hn, hn, b_bc)
                            nc.sync.dma_start(
                                out=out_d[st * 128 : (st + 1) * 128, :], in_=hn
                            )

                    return [group_step(ss, nh) for ss in range(4) for nh in range(NH)] + [tail]

                # chunk 0's qT is needed immediately; emit it directly
                xT0 = xTq_pool.tile([128, FT, 512], BF16, name="xT_q")
                for ss in range(4):
                    q_transpose_ss(xT0, 0, ss)
                q_proj_mm(0, xT0, range(FT))

                pending = []

                def pop_fill():
                    if pending:
                        pending.pop(0)()

                def emit_head(qt, h, ctx_t, den_all):
                    qsl = slice(qt * 512, (qt + 1) * 512)
                    ft, po = h // 2, (h % 2) * 64
                    ctx_ps = ps_ctx.tile([HD + 1, 512], F32, name="ctx_ps")
                    # software-pipelined: scores(g) before ctx(g-1) so the
                    # in-order PE streams scores while ACT computes exp(g-1)
                    exps = []
                    for g in range(8):
                        sc_ps = ps_sc.tile([128, 2, 512], F32, name="sc_ps")
                        for j in range(2):
                            kc = g * 2 + j
                            nc.tensor.matmul(
                                sc_ps[:, j, :],
                                kT[ft][po : po + HD, kc * 128 : (kc + 1) * 128],
                                qT[ft][po : po + HD, qsl],
                                start=True,
                                stop=True,
                            )
                        exp_g = exp_pool.tile([128, 2, 512], BF16, name="exp_g")
                        nc.scalar.activation(
                            out=exp_g, in_=sc_ps, func=AF.Exp, scale=0.125
                        )
                        exps.append(exp_g)
                        if g == 4:
                            pop_fill()  # fill PE while ACT works
                        if g > 0:
                            for j in range(2):
                                kc = (g - 1) * 2 + j
                                nc.tensor.matmul(
                                    ctx_ps,
                                    vb[kc][:, h, :],
                                    exps[g - 1][:, j, :],
                                    start=(g == 1 and j == 0),
                                    stop=False,
                                )
                    for j in range(2):
                        kc = 7 * 2 + j
                        nc.tensor.matmul(
                            ctx_ps,
                            vb[kc][:, h, :],
                            exps[7][:, j, :],
                            start=False,
                            stop=(j == 1),
                        )
                    # evict unnormalized ctx + denominator; normalization is
                    # batched per chunk, off the head loop
                    dtmp = rec_pool.tile([HD + 1, 512], F32, name="dtmp")
                    nc.vector.tensor_copy(
                        out=dtmp[HD : HD + 1, :], in_=ctx_ps[HD : HD + 1, :]
                    )
                    nc.sync.dma_start(
                        out=den_all[h : h + 1, :], in_=dtmp[HD : HD + 1, :]
                    )
                    nc.vector.tensor_copy(out=ctx_t[h], in_=ctx_ps[0:HD, :])

                def emit_norm(ctx_t, den_all):
                    # one iterative-divide pass for all 12 heads' denoms;
                    # partition-broadcast via DRAM bounce + stride-0 DMA
                    rec_all = rec_pool.tile([H, 512], F32, name="rec_all")
                    nc.vector.reciprocal(rec_all, den_all)
                    rec_d = dram_pool.tile([H, 512], F32, name="rec_d")
                    nc.sync.dma_start(out=rec_d, in_=rec_all)
                    for h in range(H):
                        bc_sb = rec_pool.tile([HD, 512], F32, name="bc_sb")
                        nc.sync.dma_start(
                            out=bc_sb,
                            in_=rec_d[h : h + 1, :].to_broadcast((HD, 512)),
                        )
                        nc.vector.tensor_mul(
                            out=ctx_t[h], in0=ctx_t[h], in1=bc_sb
                        )
                        if h % 2 == 0:
                            pop_fill()

                for qt in range(QT):
                    if qt + 1 < QT:
                        pending.extend(q_proj_steps(qt + 1))
                    ctx_t = [
                        ctx_pool.tile([HD, 512], BF16, name=f"ctx{h}")
                        for h in range(H)
                    ]
                    den_all = rec_pool.tile([H, 512], F32, name="den_all")
                    for h in range(H):
                        emit_head(qt, h, ctx_t, den_all)
                    emit_norm(ctx_t, den_all)
                    pending.extend(make_dense_steps(qt, ctx_t))
                for step in pending:
                    step()

    nc.compile()
    return nc


_NC = None


def _get_nc():
    global _NC
    if _NC is None:
        _NC = build_nc()
    return _NC


def _prepare(
    input_tensor1, attention_mask1, input_tensor2, attention_mask2,
    q1_w, q1_b, k1_w, k1_b, v1_w, v1_b,
    q2_w, q2_b, k2_w, k2_b, v2_w, v2_b,
    d1_w, d1_b, d2_w, d2_b, ln1_g, ln1_b, ln2_g, ln2_b,
):
    f = lambda a: np.ascontiguousarray(np.asarray(a), dtype=np.float32)
    x1, x2 = f(input_tensor1), f(input_tensor2)
    m1 = f(attention_mask1).reshape(B, S, 1)
    m2 = f(attention_mask2).reshape(B, S, 1)
    row = lambda a: f(a).reshape(1, HID)

    in_maps = []
    for b in range(B):
        # stream1: ctx1 = attend(q2, k1, v1, mask1); out h1[b]
        in_maps.append({
            "xq": x2[b], "xkv": x1[b],
            "wq": f(q2_w), "wk": f(k1_w), "wv": f(v1_w), "wd": f(d1_w),
            "bq": row(q2_b), "bk": row(k1_b), "bv": row(v1_b), "bd": row(d1_b),
            "mask": m1[b], "lng": row(ln1_g), "lnb": row(ln1_b),
        })
    for b in range(B):
        # stream2: ctx2 = attend(q1, k2, v2, mask2); out h2[b]
        in_maps.append({
            "xq": x1[b], "xkv": x2[b],
            "wq": f(q1_w), "wk": f(k2_w), "wv": f(v2_w), "wd": f(d2_w),
            "bq": row(q1_b), "bk": row(k2_b), "bv": row(v2_b), "bd": row(d2_b),
            "mask": m2[b], "lng": row(ln2_g), "lnb": row(ln2_b),
        })

    return in_maps


def _run(in_maps, **kwargs):
    nc = _get_nc()
    res = bass_utils.run_bass_kernel_spmd(
        nc, in_maps, core_ids=list(range(8)), **kwargs
    )
    h1 = np.stack([res.results[b]["out"] for b in range(B)])
    h2 = np.stack([res.results[B + b]["out"] for b in range(B)])
    return (h1, h2), res


def kernel(**inputs):
    (h1, h2), _ = _run(_prepare(**inputs))
    return h1, h2
